# revision 9
# baseline (speedup 1.0000x reference)
"""Trainium2 Bass kernel for the diagonal-Radon problem.

Math: the reference computes a full parallel-beam forward projection
sino[b,c,d,a] and keeps only the diagonal d==c.  So for channel j we only
need the line integral at detector offset (j-63.5) of image X[b,j]:

    out[b,j,a] = sum_t bilinear(X[b,j], u, v)
    u = 63.5 + (j-63.5)cos(th_a) - (t-63.5)sin(th_a)
    v = 63.5 + (j-63.5)sin(th_a) + (t-63.5)cos(th_a)

Device strategy (per NeuronCore, 16 channels per core, 2 passes of 8):
  - SBUF partitions = 8 GPSIMD groups (one channel each) x 16 lanes
    (2 column-shifted interleaved image copies x 8 batches).  The image is
    stored row-interleaved: block (pb,qb) of lane cs holds
    [X[pb-1, qb-1+cs], X[pb, qb-1+cs]], so ONE block index per (angle,t)
    sample fetches all 4 bilinear corners across the lanes, for all 8
    batches at once, via the extended-ISA `ap_gather` GPSIMD op (all 16
    lanes of a group share one index stream).
  - Bilinear weights (with the reference's exact validity masking and
    boundary remaps) are precomputed on the host from `angles`, replicated
    over batch lanes, DMA'd in; DVE does gathered*weight and a segmented
    t-reduction per angle.
  - Host sums the (cs, r) corner partials and reassembles [B,C,1,A].
"""

import numpy as np

N = 128
B = 8
C = 128
A = 180
C0 = np.float32(63.5)
NBLK = N * N         # 16384 blocks of 2 elements -> 32768 f32 per lane
SPP = A * N          # samples per channel = 23040
KA = 12              # angles per chunk
NCH = A // KA        # 15 chunks
NCORES = 8
JPC = 16             # channels per core
NPASS = 2
JPP = 8              # channels per pass

LAST_RESULT = None

_prog_cache = {}


def _build_program(reps=1):
    import concourse.bacc as bacc
    import concourse.mybir as mybir
    import concourse.tile as tile

    nc = bacc.Bacc("TRN2", target_bir_lowering=False, debug=False,
                   num_devices=NCORES)
    f32 = mybir.dt.float32
    i16 = mybir.dt.int16

    xs_in = [nc.dram_tensor(f"xs{h}", [128, 2 * NBLK], f32,
                            kind="ExternalInput").ap() for h in range(NPASS)]
    idx_in = [nc.dram_tensor(f"idx{h}", [128, SPP // 16], i16,
                             kind="ExternalInput").ap() for h in range(NPASS)]
    wq_in = [nc.dram_tensor(f"wq{h}", [128, SPP * 2], f32,
                            kind="ExternalInput").ap() for h in range(NPASS)]
    res_out = [nc.dram_tensor(f"res{h}", [128, A], f32,
                              kind="ExternalOutput").ap() for h in range(NPASS)]

    ns = KA * N          # samples per chunk (per channel)
    with tile.TileContext(nc) as tc:
        with tc.tile_pool(name="xsp", bufs=1) as xsp, \
             tc.tile_pool(name="idxp", bufs=2) as idxp, \
             tc.tile_pool(name="wqp", bufs=2) as wqp, \
             tc.tile_pool(name="gp", bufs=2) as gp, \
             tc.tile_pool(name="resp", bufs=2) as resp:
          for _rep in range(reps):
            for h in range(NPASS):
                xs_t = xsp.tile([128, 2 * NBLK], f32)
                nc.sync.dma_start(xs_t[:], xs_in[h])
                idx_t = idxp.tile([128, SPP // 16], i16)
                nc.sync.dma_start(idx_t[:], idx_in[h])
                res_t = resp.tile([128, A], f32)
                for k in range(NCH):
                    wq_t = wqp.tile([128, ns * 2], f32)
                    nc.sync.dma_start(
                        wq_t[:], wq_in[h][:, k * ns * 2:(k + 1) * ns * 2])
                    g_t = gp.tile([128, ns * 2], f32)
                    nc.gpsimd.ap_gather(
                        out_ap=g_t[:].rearrange("p (n d) -> p n d", d=2),
                        in_ap=xs_t[:].rearrange("p (n d) -> p n d", d=2),
                        idxs_ap=idx_t[:, k * (ns // 16):(k + 1) * (ns // 16)],
                        channels=128,
                        num_elems=NBLK,
                        d=2,
                        num_idxs=ns,
                    )
                    nc.vector.tensor_mul(g_t[:], g_t[:], wq_t[:])
                    nc.vector.tensor_reduce(
                        res_t[:, k * KA:(k + 1) * KA],
                        g_t[:].rearrange("p (a w) -> p a w", w=2 * N),
                        axis=mybir.AxisListType.X,
                        op=mybir.AluOpType.add,
                        opt_input=False,
                    )
                nc.sync.dma_start(res_out[h], res_t[:])
    nc.compile()
    return nc


def _host_tables(angles):
    """Per-(j,a,t) block indices and per-(cs,r)-corner masked bilinear
    weights.  Mirrors the reference's fp32 arithmetic order.

    Returns idx [C,A,N] int16 and W [2cs,2r,C,A,N] f32 where the (cs,r)
    corner maps to image point (pb-1+r, qb-1+cs)."""
    ang = np.asarray(angles, dtype=np.float32)
    cosv = np.cos(ang).astype(np.float32)
    sinv = np.sin(ang).astype(np.float32)
    jj = (np.arange(C, dtype=np.float32) - C0)[:, None, None]
    tt = (np.arange(N, dtype=np.float32) - C0)[None, None, :]
    cosb = cosv[None, :, None]
    sinb = sinv[None, :, None]

    u = (C0 + jj * cosb) - tt * sinb
    v = (C0 + jj * sinb) + tt * cosb
    u0 = np.floor(u)
    v0 = np.floor(v)
    wu = u - u0
    wv = v - v0
    p0 = u0.astype(np.int32)
    q0 = v0.astype(np.int32)

    pb = np.clip(p0 + 1, 0, N - 1)
    qb = np.clip(q0 + 1, 0, N - 1)
    idx = (pb * N + qb).astype(np.int16)

    one = np.float32(1.0)
    zero = np.float32(0.0)
    w = np.empty((2, 2, C, A, N), dtype=np.float32)
    for cs in range(2):
        col = qb - 1 + cs
        wcol = np.where(col == q0, one - wv, np.where(col == q0 + 1, wv, zero))
        colok = ((col >= 0) & (col < N)).astype(np.float32)
        # note: col==q0+1 only "valid" in reference if q0+1 < N, which colok
        # enforces; col==q0 needs q0 >= 0, also colok.
        wc = wcol * colok
        for r in range(2):
            row = pb - 1 + r
            wrow = np.where(row == p0, one - wu,
                            np.where(row == p0 + 1, wu, zero))
            rowok = ((row >= 0) & (row < N)).astype(np.float32)
            w[cs, r] = (wrow * rowok) * wc
    return idx, w


def _core_inputs(X, idx, w, core):
    """Build the per-core input map for chip-core `core`."""
    ins = {}
    for h in range(NPASS):
        jsel = core * JPC + h * JPP + np.arange(JPP)

        # interleaved, column-shifted image copies
        xs = np.zeros((8, 2, 8, 2 * NBLK), dtype=np.float32)  # [g,cs,b,flat]
        pad = np.zeros((B, N + 2, N + 2), dtype=np.float32)
        for g in range(JPP):
            pad[:, 1:N + 1, 1:N + 1] = X[:, jsel[g]]
            for cs in range(2):
                # flat[pb*256 + qb*2 + r] = pad[pb+r, qb+cs]
                blk = np.stack([pad[:, 0:N, cs:cs + N],
                                pad[:, 1:N + 1, cs:cs + N]], axis=-1)
                xs[g, cs] = blk.reshape(B, 2 * NBLK)
        ins[f"xs{h}"] = xs.reshape(128, 2 * NBLK)

        idxw = np.empty((8, 16, SPP // 16), dtype=np.int16)
        for g in range(JPP):
            stream = idx[jsel[g]].reshape(SPP)               # a-major
            idxw[g] = stream.reshape(SPP // 16, 16).T
        ins[f"idx{h}"] = idxw.reshape(128, SPP // 16)

        # wq[p=(g,cs,b), (a,t,r)]
        sub = w[:, :, jsel]                                  # [2cs,2r,8j,A,N]
        arr = sub.transpose(2, 0, 3, 4, 1)                   # [g,cs,A,N,r]
        arr = np.broadcast_to(arr[:, :, None], (8, 2, 8, A, N, 2))
        ins[f"wq{h}"] = np.ascontiguousarray(arr).reshape(128, SPP * 2)
    return ins


def kernel(X, angles):
    global LAST_RESULT
    import os
    # No NTFF/axon profiling hook in this environment; make sure a stray
    # BASS_TRACE=1 can't route us into the missing antenv.axon_hooks import.
    os.environ["BASS_NEVER_TRACE"] = "1"
    from concourse.bass_utils import run_bass_kernel_spmd

    X = np.ascontiguousarray(np.asarray(X, dtype=np.float32))
    if "nc" not in _prog_cache:
        _prog_cache["nc"] = _build_program()
    nc = _prog_cache["nc"]

    akey = np.asarray(angles, dtype=np.float32).tobytes()
    if _prog_cache.get("akey") != akey:
        _prog_cache["tables"] = _host_tables(angles)
        _prog_cache["akey"] = akey
    idx, w = _prog_cache["tables"]
    in_maps = [_core_inputs(X, idx, w, c) for c in range(NCORES)]
    _prog_cache["in_maps"] = in_maps

    result = run_bass_kernel_spmd(
        nc, in_maps, core_ids=list(range(NCORES)), trace=False)
    LAST_RESULT = result

    out = np.zeros((B, C, 1, A), dtype=np.float32)
    for c in range(NCORES):
        for h in range(NPASS):
            res = result.results[c][f"res{h}"].reshape(8, 2, 8, A)  # [g,cs,b,A]
            part = res[:, 0] + res[:, 1]                            # [g,b,A]
            jsel = c * JPC + h * JPP + np.arange(JPP)
            out[:, jsel, 0, :] = part.transpose(1, 0, 2)
    return out


# ---------------------------------------------------------------------------
# Timing support (no NTFF profiling hook in this environment): slope method.
# ---------------------------------------------------------------------------

def _make_sharded_callable(nc):
    import jax
    from jax.sharding import Mesh, PartitionSpec, NamedSharding
    from jax.experimental.shard_map import shard_map
    import concourse.mybir as mybir
    import concourse.bass2jax as bass2jax

    bass2jax.install_neuronx_cc_hook()

    partition_name = (nc.partition_id_tensor.name
                      if nc.partition_id_tensor else None)
    in_names, out_names, out_avals, zero_outs = [], [], [], []
    for alloc in nc.m.functions[0].allocations:
        if not isinstance(alloc, mybir.MemoryLocationSet):
            continue
        name = alloc.memorylocations[0].name
        if alloc.kind == "ExternalInput":
            if name != partition_name:
                in_names.append(name)
        elif alloc.kind == "ExternalOutput":
            out_names.append(name)
            shape = tuple(alloc.tensor_shape)
            dtype = mybir.dt.np(alloc.dtype)
            out_avals.append(jax.core.ShapedArray(shape, dtype))
            zero_outs.append(np.zeros(shape, dtype))
    n_params = len(in_names)
    all_in_names = list(in_names) + list(out_names)
    if partition_name is not None:
        all_in_names.append(partition_name)

    def _body(*args):
        operands = list(args)
        if partition_name is not None:
            operands.append(bass2jax.partition_id_tensor())
        outs = bass2jax._bass_exec_p.bind(
            *operands,
            out_avals=tuple(out_avals),
            in_names=tuple(all_in_names),
            out_names=tuple(out_names),
            lowering_input_output_aliases=(),
            sim_require_finite=True,
            sim_require_nnan=True,
            nc=nc,
        )
        return tuple(outs)

    devices = jax.devices()[:NCORES]
    mesh = Mesh(np.asarray(devices), ("core",))
    spec = PartitionSpec("core")
    in_specs = (spec,) * (n_params + len(out_names))
    out_specs = (spec,) * len(out_names)
    donate = tuple(range(n_params, n_params + len(out_names)))
    fn = jax.jit(
        shard_map(_body, mesh=mesh, in_specs=in_specs, out_specs=out_specs,
                  check_rep=False),
        donate_argnums=donate, keep_unused=True)
    sharding = NamedSharding(mesh, spec)
    return fn, in_names, zero_outs, sharding


def _timed_exec(nc, in_maps, iters):
    import time
    import jax

    fn, in_names, zero_outs, sharding = _make_sharded_callable(nc)
    concat_in = [
        jax.device_put(
            np.concatenate([np.asarray(in_maps[c][n]) for c in range(NCORES)],
                           axis=0), sharding)
        for n in in_names
    ]

    def one_call():
        zeros = [
            jax.device_put(
                np.zeros((NCORES * z.shape[0], *z.shape[1:]), z.dtype),
                sharding)
            for z in zero_outs
        ]
        for z in zeros:
            z.block_until_ready()
        t0 = time.monotonic()
        outs = fn(*concat_in, *zeros)
        for o in outs:
            o.block_until_ready()
        return time.monotonic() - t0

    one_call()  # compile + warm
    times = [one_call() for _ in range(iters)]
    return float(np.median(times)), times


def measure_hw_time_ns(iters=25, reps=17):
    """Estimated on-device exec time via the slope method."""
    nc1 = _prog_cache.get("nc")
    in_maps = _prog_cache.get("in_maps")
    if nc1 is None or in_maps is None:
        raise RuntimeError("run kernel() first")
    key = f"ncR{reps}"
    if key not in _prog_cache:
        _prog_cache[key] = _build_program(reps=reps)
    ncR = _prog_cache[key]
    _, t1_all = _timed_exec(nc1, in_maps, iters)
    _, tR_all = _timed_exec(ncR, in_maps, iters)
    t1 = min(t1_all)
    tR = min(tR_all)
    est = (tR - t1) / (reps - 1)
    return (est * 1e9, t1 * 1e9, tR * 1e9,
            [t * 1e9 for t in t1_all], [t * 1e9 for t in tR_all])
